# revision 3
# baseline (speedup 1.0000x reference)
"""Trainium2 Bass kernel v2 for nn_Conv2D_BinaryLayer — weight-stationary.

  x      [32, 112, 112, 128] f32  (NHWC)
  kernel [3, 3, 128, 256]    f32  -> binarized on host to {-1, +1}
  bias   [256]               f32
  out    [32, 110, 110, 256] f32

Data-parallel over batch, 4 images per core on 8 cores. Per core the conv
is an implicit GEMM; v2 makes the WEIGHTS stationary and streams x as the
moving operand, transposing the output to [co, pos] (host untransposes).

Why: the baseline (x-stationary) pays 7 LDWEIGHTS per 128-position block
(757ns) against 763ns of matmul streaming — the LDW port sits at 94% and
is the serialized resource (297us busy of a 329us wall). Weight-stationary
amortizes each stationary over a 440-position chunk: LDW drops to ~80% of
stream and the stream itself (14 cyc/pos) becomes the binder.

Precision mix (same as baseline, measured rel err 1.74e-2 < 2e-2 gate):
  - taps (0,0),(1,0) and (0,1),(1,1) as TWO fp8 DoubleRowSwInterleave
    pair-matmuls per 220-pos sub-block: stationary = host-interleaved
    binary weights [ci, 2*(127-q)+ko] (q = cout-in-half, ko = kh), moving
    = x8 as a strided 3D AP [ci, ko(step 112), pos(step 1)] — emitted with
    a placeholder step 220 (bass can't express overlapping dims) and
    patched to 112 post-build. 112 % 16 == 0 satisfies the ISA's
    dual-fp8 moving-AP constraint.
  - the other 5 taps as bf16 matmuls, moving = xb[base+t : base+t+440].
Bias is fused into the PSUM->SBUF drain on the SCALAR engine (ACT
Identity, per-partition bias = bias[co]), which also downcasts to bf16 —
halving store traffic vs f32 (sim: rel err 1.74e-2 unchanged). DVE idle.

Per (image, chunk=440 pos, cout-half): 4 DR + 5 bf16 matmuls accumulate
into one PSUM tile [128, 440] f32; the DVE drains into a 4-chunk osb
batch; one 1760-pos store per batch (7 stores per image-half,
[128 x 3.4KB]). Loads ride the ACT HWDGE ring (8 chunks per image per
dtype, prefetch distance 1 in quarter-bursts), stores the SP ring.

Scheduling findings baked in (each measured on HW):
  - DR<->plain perf-mode transitions cost ~55ns each (they defeat the PE
    queue's LDWEIGHTS lookahead; fully alternating ran 74us slower), so
    all DR matmuls of a chunk PAIR are emitted together, then all bf16.
  - The PSUM drain lives on DVE, keeping the Scalar queue free for load
    DMA issues (sharing them stalled the drain behind ~13us of
    descriptor generation and backed up PSUM). GpSimd DMA is the slow
    software-DGE path; the SP ring stalls on ring depth at startup.
  - 16 warmup matmuls on a memset scratch tile ramp the HAM clock gate
    to full rate before the first real matmul.
  - bf16 output stores halve store traffic; the host widens to f32
    (sim + HW: rel err 1.7249e-2, unchanged vs f32 stores).

Measured: 312-314us HW exec (baseline x-stationary kernel: 323us), PE
matmul stream is the binding resource at ~290us (14 cyc/pos); LDW port
~75%, DVE ~52%, edges ~11us startup + ~11us drain/barrier tail.
"""

import numpy as np
from contextlib import ExitStack

import concourse.bass as bass
import concourse.tile as tile
from concourse import mybir
from concourse import bass_utils
from concourse.bass_utils import run_bass_kernel_spmd

_orig_bir_opt = bass_utils.bir_verify_and_optimise


def _bir_opt_ldw(tmpdir, inp="bir.json", outp="file.neff", arch=None, *,
                 dve_root=None):
    """bass_utils.bir_verify_and_optimise with --enable-ldw-opt=true:
    lets walrus fuse/background the plain bf16 LDWEIGHTS (FWL). The
    DoubleRow* matmuls are rejected by the LDW optimizer and keep their
    explicit loads, same as with the flag off."""
    from pathlib import Path
    from concourse.bass_utils import (
        get_walrus_driver, get_walrus_args, get_bir_arch, run_command)
    from concourse.aot_env import aot_checkenv, aot_getenv
    cmd = [
        get_walrus_driver(),
        "--pass",
        ",".join([
            "birverifier", "runtime_memory_reservation", "lower_act",
            "lower_dve", "lower_ap_offset", "codegen", "neff_packager",
        ]),
        "-i", inp,
        "--neff-output-filename", outp,
        "--enable-birsim=true", "--mem-mode=physical", "--policy=0",
        "--enable-ldw-opt=true", "--assign-static-dmas-to-sp=false",
        f"--dram-page-size={aot_getenv('NEURON_SCRATCHPAD_PAGE_SIZE', '256')}",
        f"--enable-neff-debug-info={'false' if aot_checkenv('CONCOURSE_SCRUB_NEFF_DEBUG_INFO') else 'true'}",
        "--jobs", "8",
        *get_walrus_args(
            get_bir_arch(tmpdir, inp) if arch is None else arch,
            tmpdir, dve_root=dve_root),
    ]
    result = run_command(cmd, cwd=tmpdir)
    if result is not None:
        (Path(tmpdir) / "log.txt").write_text(result.stdout)
    return f"{tmpdir}/{outp}"


USE_LDW_OPT = False  # walrus's LDW optimizer errors out on DoubleRow loads

# ---------------------------------------------------------------- shapes
N, H, W, CIN, COUT = 32, 112, 112, 128, 256
KH = KW = 3
HO, WO = H - KH + 1, W - KW + 1      # 110, 110
N_CORES = 8
NPC = N // N_CORES                   # images per core = 4
PIX = H * W                          # 12544
NPOS = HO * W                        # 12320 flat grid positions per image
IMG = 12800                          # x stride per image (12544 + pad)
CH = 440                             # positions per chunk
NCH = NPOS // CH                     # 28 chunks per image
SUB = CH // 2                        # 220, DR sub-block
OSB_B = 4                            # chunks batched per output store
DR_KW = (0, 1)                       # DR pairs: taps (kw, 112+kw)
BF_TAPS = (2, 114, 224, 225, 226)    # bf16 taps (kw=2 col + kh=2 row)
NHALF = 2                            # cout halves

_F32 = mybir.dt.float32
_BF16 = mybir.dt.bfloat16
_F8 = mybir.dt.float8e4


def _split_waits(nc, maxw=1):
    """walrus in this container rejects multiple sync-waits per instruction.
    Move overflow waits onto NoOps inserted just before the instruction."""
    for f in nc.m.functions:
        for bb in f.blocks:
            new_insts = []
            for inst in bb.instructions:
                si = inst.sync_info
                if si is not None and si.on_wait and len(si.on_wait) > maxw:
                    waits = list(si.on_wait)
                    overflow, keep = waits[:-maxw], waits[-maxw:]
                    for ci in range(0, len(overflow), 1):
                        nop = mybir.InstNoOp(
                            name=f"{inst.name}-ws{ci}",
                            engine=inst.engine,
                            ins=[], outs=[],
                            sync_info=mybir.SyncInfo(
                                on_wait=overflow[ci:ci + 1], on_update=[]),
                        )
                        nc.register_instruction(nop, overwrite=True)
                        new_insts.append(nop)
                    inst.sync_info = mybir.SyncInfo(
                        on_wait=keep, on_update=list(si.on_update or []))
                new_insts.append(inst)
            bb.instructions[:] = new_insts


def build_nc():
    nc = bass.Bass("TRN2", target_bir_lowering=False, debug=False,
                   num_devices=N_CORES, num_swdge_queues=2)

    xb_d = nc.dram_tensor("xb", [CIN, NPC * IMG], _BF16, kind="ExternalInput")
    x8_d = nc.dram_tensor("x8", [CIN, NPC * IMG], _F8, kind="ExternalInput")
    si_d = nc.dram_tensor("si", [CIN, len(DR_KW) * NHALF * 256], _F8,
                          kind="ExternalInput")
    bw_d = nc.dram_tensor("bw", [CIN, len(BF_TAPS) * NHALF * 128], _BF16,
                          kind="ExternalInput")
    b_d = nc.dram_tensor("bias2", [128, NHALF], _F32, kind="ExternalInput")
    o_d = nc.dram_tensor("out", [NPC, NHALF, 128, NPOS], _BF16,
                         kind="ExternalOutput")

    dr_mm_names = []

    with tile.TileContext(nc) as tc, ExitStack() as ctx:
        const_pool = ctx.enter_context(tc.tile_pool(name="const", bufs=1))
        xb_pool = ctx.enter_context(tc.tile_pool(name="xb", bufs=2))
        x8_pool = ctx.enter_context(tc.tile_pool(name="x8", bufs=2))
        osb_pool = ctx.enter_context(tc.tile_pool(name="osb", bufs=2))
        ps_pool = ctx.enter_context(
            tc.tile_pool(name="ps", bufs=4, space="PSUM"))

        # --- constants ---------------------------------------------------
        si_sb = const_pool.tile([CIN, len(DR_KW), NHALF, 256], _F8, tag="si")
        nc.sync.dma_start(si_sb[:].rearrange("p a b c -> p (a b c)"),
                          si_d.ap()[:])
        bw_sb = const_pool.tile([CIN, len(BF_TAPS), NHALF, 128], _BF16,
                                tag="bw")
        nc.sync.dma_start(bw_sb[:].rearrange("p a b c -> p (a b c)"),
                          bw_d.ap()[:])
        bias_sb = const_pool.tile([128, NHALF], _F32, tag="bias")
        nc.sync.dma_start(bias_sb[:], b_d.ap()[:])

        # PE warmup: ~16 dummy matmuls on a memset scratch tile, issued
        # while the first x loads are in flight. The HAM clock gate needs
        # ~3us of continuous PE activity to reach full rate - without this
        # the first real matmuls run at the mid p-state.
        warm = const_pool.tile([128, 512], _BF16, tag="warm")
        nc.vector.memset(warm[:], 0)
        wps = ps_pool.tile([128, CH], _F32, tag="ps0", name="warmps")
        for wi in range(16):
            nc.tensor.matmul(wps[:], warm[:, :128], warm[:, :440],
                             start=(wi == 0), stop=(wi == 15),
                             skip_group_check=True)

        # --- per-image x tiles, prefetch distance 1 ----------------------
        xbs, x8s = {}, {}
        NLD = 8                       # load chunks per image per dtype
        LD = PIX // NLD               # 1568 elems per load chunk
        # steady-state images load in 4 double-chunks: fewer DMA
        # completion semaphores for the PE queue to wait on

        def load_image(n, quarter=None):
            if n >= NPC:
                return
            if quarter in (None, 0):
                xbt = xb_pool.tile([CIN, IMG], _BF16, tag="xb")
                x8t = x8_pool.tile([CIN, IMG], _F8, tag="x8")
                xbs[n], x8s[n] = xbt, x8t
                nc.vector.memset(xbt[:, PIX:IMG], 0)
                nc.vector.memset(x8t[:, PIX:IMG], 0)
            xbt, x8t = xbs[n], x8s[n]
            q = NLD // 4
            js = ([(j, j + 1) for j in range(NLD)] if quarter is None else
                  [(j, j + 1) for j in range(quarter * q, (quarter + 1) * q)])
            # all loads ride the ACT (scalar) HWDGE ring - the fastest issue
            # path (SP stalls on ring depth, GpSimd is the slow software
            # DGE). The PSUM drain lives on DVE so nothing queues behind
            # these issues.
            for (j, je) in js:
                a, b = j * LD, je * LD
                if n == 0 and j == 0:
                    # split the first chunk: chunk-0 compute only needs
                    # x8[0:552] / xb[0:667], so a 784-elem first transfer
                    # lets the PE start ~0.5us sooner
                    h0 = LD // 2
                    for (lo, hi) in ((0, h0), (h0, b)):
                        nc.scalar.dma_start(x8t[:, lo:hi],
                                            x8_d.ap()[:, lo:hi])
                        nc.scalar.dma_start(xbt[:, lo:hi],
                                            xb_d.ap()[:, lo:hi])
                    continue
                nc.scalar.dma_start(x8t[:, a:b],
                                    x8_d.ap()[:, n * IMG + a:n * IMG + b])
                nc.scalar.dma_start(xbt[:, a:b],
                                    xb_d.ap()[:, n * IMG + a:n * IMG + b])

        load_image(0)
        for n in range(NPC):
            load_image(n + 1, quarter=0)
            xbt, x8t = xbs.pop(n), x8s.pop(n)
            osb = [None, None]
            for cc in range(0, NCH, 2):
                # minimize DR<->plain perf-mode transitions: alternating
                # modes per-matmul measured 74us SLOWER (~55ns a switch —
                # it defeats the PE queue's LDWEIGHTS lookahead). Emit ALL
                # DR matmuls of a PAIR of chunks (both cout halves), then
                # all the bf16 ones: 1 transition per chunk.
                pair = (cc, cc + 1)
                pscs = {c: [ps_pool.tile([128, CH], _F32, tag=f"ps{h}",
                                         name=f"ps{h}")
                            for h in range(NHALF)] for c in pair}
                for c in pair:
                    cb = c * CH
                    for half in range(NHALF):
                        for s in range(2):
                            sb = cb + s * SUB
                            for pi, kw in enumerate(DR_KW):
                                rhs = x8t[:, sb + kw:sb + kw + 2 * SUB
                                          ].rearrange("p (a b) -> p a b", a=2)
                                mm = nc.tensor.matmul(
                                    pscs[c][half][:, s * SUB:(s + 1) * SUB],
                                    si_sb[:, pi, half, :], rhs,
                                    start=(s == 0 and pi == 0), stop=False,
                                    perf_mode=(mybir.MatmulPerfMode
                                               .DoubleRowSwInterleave),
                                    skip_group_check=True)
                                dr_mm_names.append(mm.ins.name)
                for c in pair:
                    cb = c * CH
                    for half in range(NHALF):
                        for ti in range(len(BF_TAPS)):
                            t = BF_TAPS[ti]
                            nc.tensor.matmul(
                                pscs[c][half][:], bw_sb[:, ti, half, :],
                                xbt[:, cb + t:cb + t + CH],
                                start=False, stop=(ti == len(BF_TAPS) - 1),
                                skip_group_check=True)
                for c in pair:
                    cb = c * CH
                    for half in range(NHALF):
                        # last image stores in half-size batches so the
                        # final store drain (and the end-of-NEFF barrier
                        # that waits on it) is shorter
                        B = OSB_B if n < NPC - 1 else OSB_B // 2
                        m = c % B
                        if m == 0:
                            osb[half] = osb_pool.tile(
                                [128, B * CH], _BF16,
                                tag=f"osb{half}_{B}", name=f"osb{half}")
                        # drain on DVE (idle otherwise): bias + bf16 cast
                        nc.vector.tensor_scalar_add(
                            osb[half][:, m * CH:(m + 1) * CH],
                            pscs[c][half][:],
                            bias_sb[:, half:half + 1])
                        if m == B - 1:
                            nc.sync.dma_start(
                                o_d.ap()[n, half, :, cb - (B - 1) * CH:
                                         cb + CH],
                                osb[half][:])
                    if c in (7, 14, 21):
                        load_image(n + 1, quarter=c // 7)

    # --- patch DR moving APs: pair-dim step 220 -> 112 (overlapping AP,
    # inexpressible through the tile API; the emitted range over-covers the
    # true reads so scheduling deps stay safe) --------------------------
    names = set(dr_mm_names)
    patched = 0
    for f in nc.m.functions:
        for bb in f.blocks:
            for inst in bb.instructions:
                if isinstance(inst, mybir.InstMatmult) and inst.name in names:
                    ap = inst.ins[0].ap
                    assert list(ap[1]) == [SUB, 2], (inst.name, ap)
                    assert list(ap[2]) == [1, SUB], (inst.name, ap)
                    ap[1] = [112, 2]
                    patched += 1
    assert patched == len(dr_mm_names), (patched, len(dr_mm_names))

    _split_waits(nc)
    return nc


_NC_CACHE = None

# test knobs (same contract as the baseline kernel)
TRACE = False
TRACE_KW: dict = {}
LAST_RESULTS = None


def _get_nc():
    global _NC_CACHE
    if _NC_CACHE is None:
        _NC_CACHE = build_nc()
    return _NC_CACHE


def kernel(x: np.ndarray, kernel: np.ndarray, bias: np.ndarray) -> np.ndarray:
    global LAST_RESULTS
    import ml_dtypes
    nc = _get_nc()

    # binarize on host, matching fp32 ref semantics:
    #   wb = +1  iff  fl(w + 1.0) > 1.0  else -1
    kb = np.where((kernel.astype(np.float32) + np.float32(1.0))
                  > np.float32(1.0), np.float32(1.0), np.float32(-1.0))

    # DR stationaries, SwInterleave layout: byte col 2*(127-q)+ko of pair
    # (kw, 112+kw) half h holds kb[kh=ko, kw, ci, 128h+q]
    si = np.empty((CIN, len(DR_KW), NHALF, 256), dtype=ml_dtypes.float8_e4m3)
    q = np.arange(128)
    for pi, kw in enumerate(DR_KW):
        for h in range(NHALF):
            for ko in range(2):
                si[:, pi, h, 2 * (127 - q) + ko] = kb[ko, kw, :, 128 * h + q].T
    si = np.ascontiguousarray(si.reshape(CIN, -1))

    # bf16 stationaries, natural [ci, co] layout per (tap, half)
    bw = np.empty((CIN, len(BF_TAPS), NHALF, 128), dtype=ml_dtypes.bfloat16)
    for ti, t in enumerate(BF_TAPS):
        kh, kw = divmod(t, W)
        for h in range(NHALF):
            bw[:, ti, h, :] = kb[kh, kw, :, 128 * h:128 * (h + 1)]
    bw = np.ascontiguousarray(bw.reshape(CIN, -1))

    bias2 = np.ascontiguousarray(
        bias.astype(np.float32).reshape(NHALF, 128).T)

    # x: [n,h,w,ci] f32 -> per-core [ci, n*IMG] in bf16 and fp8
    xb = x.astype(ml_dtypes.bfloat16)
    x8 = x.astype(ml_dtypes.float8_e4m3)
    in_maps = []
    for c in range(N_CORES):
        xbc = np.zeros((CIN, NPC * IMG), dtype=ml_dtypes.bfloat16)
        x8c = np.zeros((CIN, NPC * IMG), dtype=ml_dtypes.float8_e4m3)
        for n in range(NPC):
            img = c * NPC + n
            xbc[:, n * IMG:n * IMG + PIX] = xb[img].reshape(PIX, CIN).T
            x8c[:, n * IMG:n * IMG + PIX] = x8[img].reshape(PIX, CIN).T
        in_maps.append({"xb": xbc, "x8": x8c, "si": si, "bw": bw,
                        "bias2": bias2})

    bass_utils.bir_verify_and_optimise = (
        _bir_opt_ldw if USE_LDW_OPT else _orig_bir_opt)
    try:
        res = run_bass_kernel_spmd(nc, in_maps, list(range(N_CORES)),
                                   trace=TRACE, **TRACE_KW)
    finally:
        bass_utils.bir_verify_and_optimise = _orig_bir_opt
    LAST_RESULTS = res

    parts = []
    for c in range(N_CORES):
        o = res.results[c]["out"]  # [NPC, 2, 128, NPOS] bf16
        o = np.asarray(o).astype(np.float32)
        # [n, half, q, pos] -> [n, pos, half*128+q] -> [n, 110, 112, 256]
        o = o.transpose(0, 3, 1, 2).reshape(NPC, HO, W, COUT)[:, :, :WO, :]
        parts.append(o)
    return np.ascontiguousarray(np.concatenate(parts, axis=0),
                                dtype=np.float32)
